# revision 36
# baseline (speedup 1.0000x reference)
"""DINOPPNet Trainium2 kernel — 8-core data-parallel over batch.

Computation (per the nn.Module):
    distances[b,n,p] = relu(||x_bn||^2 - 2 x_bn.p_p + ||p_p||^2)
    attn_maps        = distances.transpose -> [B, P, N]
    min_distances    = min over n          -> [B, P]
    act              = log((m+1)/(m+eps))
    logits           = act @ fc_w.T        -> [B, C]

Strategy:
  - Data parallel: batch B=64 split 8 ways (8 images/core). Prototypes and
    fc weights replicated.
  - Per core, per batch image, the main matmul is computed prototype-major:
    G[p, n] = sum_d protoT[d,p] * (-2*patch)T[d,n], so the PSUM/SBUF tile is
    already in attn_maps layout [P, N] (no output transpose needed).
  - Operands are bf16 (fp32 accumulation in PSUM): operand-rounding errors
    random-walk over K=1024 to ~3e-4 relative on the outputs.
  - The relu is a mathematical no-op for these inputs (L2 distances ~1100+),
    so PSUM holds -2*x.p directly; the ||p||^2 term is added as a per-
    partition fp32 ScalarE bias during PSUM evacuation and the ||x||^2 term
    by a DVE add of a host-broadcast fp32 tile. Both corrections are exact.
  - min over n via DVE free-dim reduce; activation via DVE reciprocal +
    ln(1+s) quartic series (avoids ScalarE LUT precision concerns);
    logits via 16 accumulated [K=128] x [8, 200] bf16 matmuls; min_distances
    transposed to [b, p] with 16 PE identity-transposes.
  - The instruction schedule keeps the PE dense from its first matmul: the
    input DMA order delivers (prototype-head, patch-chunk) pairs at ~1.65us
    each while phase A (p-tiles 0/1, k-outer across all 8 PSUM banks)
    consumes chunks at ~1.7us, then mid/tail prototype columns stream in the
    DMA slack behind phases B/C. Activation/transpose/logits work is
    interleaved into the main loop so the kernel tail is short.

Host-side prep (sharding + layout only): transposes to put the contraction
dim on partitions, bf16 casts, zero-padding P 2000->2048, and the p2/x2
sum-of-squares vectors. The 67 GFLOP matmul, min-reduce, activation and fc
all run on device. Cost-model (TimelineSim) end-to-end: ~124us/core;
PE busy ~112.5us vs the 109.2us bf16 matmul roofline.
"""

from contextlib import ExitStack

import numpy as np
import ml_dtypes

import concourse.bacc as bacc
import concourse.tile as tile
import concourse.mybir as mybir
from concourse.bass_utils import run_bass_kernel_spmd

BF16 = ml_dtypes.bfloat16

B, N, D = 64, 256, 1024
P, C = 2000, 200
EPS = 1e-4

NCORES = 8
NB = B // NCORES            # batches per core = 8
COLS = NB * N               # matmul moving columns per core = 2048
PPAD = 2048                 # prototypes padded to 16*128
PT = PPAD // 128            # p tiles = 16
KD = D // 128               # contraction chunks = 8
GF = 512                    # moving free dim per matmul (one PSUM bank of fp32)
G = COLS // GF              # column groups = 4

F32 = mybir.dt.float32
BF = mybir.dt.bfloat16


def _emit(ctx, tc, aps, reps):
    nc = tc.nc
    patT, protT, x2bc, p2col, fcwT, ident = (
        aps["patT"], aps["protT"], aps["x2bc"], aps["p2col"],
        aps["fcwT"], aps["ident"],
    )
    attn, mind, logits = aps["attn"], aps["mind"], aps["logits"]

    consts = ctx.enter_context(tc.tile_pool(name="consts", bufs=1))
    dist_pool = ctx.enter_context(tc.tile_pool(name="dist", bufs=6))
    psum_pool = ctx.enter_context(tc.tile_pool(name="psum", bufs=7, space="PSUM"))
    psum_l = ctx.enter_context(tc.tile_pool(name="psum_l", bufs=1, space="PSUM"))
    misc = ctx.enter_context(tc.tile_pool(name="misc", bufs=2))

    # ---- input DMA schedule (single queue => deterministic arrivals) ----
    prot_sb = [consts.tile([128, PPAD], BF, name=f"prot{k}") for k in range(KD)]
    pat_sb = [consts.tile([128, COLS], BF, name=f"pat{k}") for k in range(KD)]
    # (head_k, pat_k) pairs: phase A (pt0/pt1, k-outer over all 8 psum
    # banks) consumes chunk k at ~1.7us while pair k+1 arrives in ~1.65us,
    # so the PE stream stays dense from the very first matmul.
    for k in range(KD):
        nc.sync.dma_start(prot_sb[k][:, :256],
                          protT[k * 128:(k + 1) * 128, :256])
        nc.sync.dma_start(pat_sb[k][:], patT[k * 128:(k + 1) * 128, :])
    # everything else streams in the slack behind the PE: p2 before the
    # first evacuation, mid prototype columns before phase B (pt2/pt3),
    # the host-broadcast x2 tile before the first DVE add, tail columns
    # before phase C, fc weights / identity long before the kernel tail
    p2_sb = consts.tile([128, PT], F32)
    nc.sync.dma_start(p2_sb[:], p2col[:])
    for k in range(KD):
        nc.sync.dma_start(prot_sb[k][:, 256:512],
                          protT[k * 128:(k + 1) * 128, 256:512])
    x2b = consts.tile([128, COLS], F32)
    nc.sync.dma_start(x2b[:], x2bc[:])
    for k in range(KD):
        nc.sync.dma_start(prot_sb[k][:, 512:],
                          protT[k * 128:(k + 1) * 128, 512:])
    fcw_sb = consts.tile([128, PT, C], BF)
    nc.sync.dma_start(fcw_sb[:], fcwT.rearrange("(t p) c -> p t c", p=128))
    ident_sb = consts.tile([128, 128], F32)
    nc.sync.dma_start(ident_sb[:], ident[:])

    for rep in range(reps):
        mind_sb = misc.tile([128, PT, NB], F32, tag="mind")
        act_bf = misc.tile([128, PT, NB], BF, tag="act")

        def act_chain(sl):
            """a = ln(1 + s), s = (1-eps)/(m+eps), via quartic ln1p series.

            Runs on a [128, len(sl)*NB] column slice of mind_sb so the tail
            half overlaps the main loop.
            """
            mf = mind_sb[:, sl].rearrange("p t b -> p (t b)")
            af = act_bf[:, sl].rearrange("p t b -> p (t b)")
            w = len(range(*sl.indices(PT))) * NB
            u = misc.tile([128, w], F32, tag="u")
            s = misc.tile([128, w], F32, tag="s")
            a = misc.tile([128, w], F32, tag="a")
            nc.vector.tensor_scalar_add(u[:], mf, EPS)
            nc.vector.reciprocal(s[:], u[:])
            nc.vector.tensor_scalar_mul(s[:], s[:], 1.0 - EPS)
            nc.vector.tensor_scalar(u[:], s[:], -0.25, 1.0 / 3.0,
                                    mybir.AluOpType.mult, mybir.AluOpType.add)
            nc.vector.tensor_tensor(u[:], s[:], u[:], mybir.AluOpType.mult)
            nc.vector.tensor_scalar(u[:], u[:], -1.0, 0.5,
                                    mybir.AluOpType.mult, mybir.AluOpType.add)
            nc.vector.tensor_tensor(u[:], s[:], u[:], mybir.AluOpType.mult)
            nc.vector.tensor_scalar(u[:], u[:], -1.0, 1.0,
                                    mybir.AluOpType.mult, mybir.AluOpType.add)
            nc.vector.tensor_tensor(a[:], s[:], u[:], mybir.AluOpType.mult)
            nc.vector.tensor_copy(af, a[:])

        # ---- main loop: distances per p-tile ----
        gsl = lambda g: slice(g * GF, (g + 1) * GF)

        def mm(ps, pt, g, k, start, stop):
            nc.tensor.matmul(
                ps, prot_sb[k][:, pt * 128:(pt + 1) * 128],
                pat_sb[k][:, gsl(g)], start=start, stop=stop)

        dists = {}

        def new_dist(pt):
            dists[pt] = dist_pool.tile([128, COLS], F32, tag="dist",
                                       name=f"dist_{rep}_{pt}")
            return dists[pt]

        def evac_first(pt, g, ps):
            # dist = ps + p2[p]  (fp32-exact per-partition bias)
            nc.scalar.activation(
                dists[pt][:, gsl(g)], ps,
                mybir.ActivationFunctionType.Identity,
                bias=p2_sb[:, pt:pt + 1], scale=1.0)

        def add_psum(pt, g, ps):
            nc.vector.tensor_tensor(
                dists[pt][:, gsl(g)], dists[pt][:, gsl(g)], ps,
                mybir.AluOpType.add)

        def add_x2(pt, g):
            nc.vector.tensor_tensor(
                dists[pt][:, gsl(g)], dists[pt][:, gsl(g)], x2b[:, gsl(g)],
                mybir.AluOpType.add)

        def finish_tile(pt):
            # min over n within each batch: [128, 8, 256] -> [128, 8]
            nc.vector.tensor_reduce(
                mind_sb[:, pt],
                dists[pt].rearrange("p (b n) -> p b n", b=NB),
                axis=mybir.AxisListType.X,
                op=mybir.AluOpType.min,
            )
            psz = 128 if pt < PT - 1 else P - 128 * (PT - 1)
            nc.sync.dma_start(
                attn[:, pt * 128:pt * 128 + psz, :].rearrange("b p n -> p b n"),
                dists[pt][:psz].rearrange("p (b n) -> p b n", b=NB),
            )

        # phase A: pt0 + pt1 full-K, k-outer across all 8 psum banks,
        # consuming pat chunks as the DMAs land (the prototype head columns
        # are tiny and arrive before the pat stream)
        for pt in range(2):
            new_dist(pt)
        psA = [psum_pool.tile([128, GF], F32, tag="acc",
                              name=f"r{rep}A0g{g}") for g in range(G)]
        psA += [psum_pool.tile([128, GF], F32, tag="acc",
                               name=f"r{rep}A1g{g}") for g in range(G - 1)]
        psA.append(psum_l.tile([128, GF], F32, tag="psl", name=f"r{rep}A1g3"))
        for k in range(KD):
            for pt in range(2):
                for g in range(G):
                    mm(psA[pt * G + g], pt, g, k,
                       start=(k == 0), stop=(k == KD - 1))
        for pt in range(2):
            for g in range(G):
                evac_first(pt, g, psA[pt * G + g])
                add_x2(pt, g)
            finish_tile(pt)
        # phases B/C: pt2..15 full-K, g-inner (pt2/pt3 need only the early
        # mid columns; later tiles consume the tail stream)
        mindT = misc.tile([NB, PPAD], F32, tag="mindT")
        for pt in range(2, PT):
            dist = new_dist(pt)
            last = pt == PT - 1
            for g in range(G):
                ps = psum_pool.tile([128, GF], F32, tag="acc")
                for k in range(KD):
                    mm(ps, pt, g, k, start=(k == 0), stop=(k == KD - 1))
                evac_first(pt, g, ps)
                add_x2(pt, g)
                if last:
                    # per-group min so the tail isn't serialized behind one
                    # whole-tile reduce
                    nc.vector.tensor_reduce(
                        mind_sb[:, pt, 2 * g:2 * g + 2],
                        dist[:, gsl(g)].rearrange("p (b n) -> p b n", b=2),
                        axis=mybir.AxisListType.X,
                        op=mybir.AluOpType.min,
                    )
            if last:
                psz = P - 128 * (PT - 1)
                nc.sync.dma_start(
                    attn[:, pt * 128:pt * 128 + psz, :].rearrange(
                        "b p n -> p b n"),
                    dist[:psz].rearrange("p (b n) -> p b n", b=NB),
                )
            else:
                finish_tile(pt)
            if pt == PT // 2 - 1:
                act_chain(slice(0, PT // 2))
            if pt == 10:
                # transpose the first half of min_distances while the main
                # loop still has work: [128, 8] tiles -> [8, 128] each
                for tpt in range(PT // 2):
                    pst = psum_pool.tile([NB, 128], F32, tag="acc",
                                         name=f"r{rep}tp{tpt}")
                    nc.tensor.transpose(pst, mind_sb[:, tpt], ident_sb[:])
                    nc.scalar.copy(mindT[:, tpt * 128:(tpt + 1) * 128], pst)
            if pt == 11:
                nc.sync.dma_start(mind[:, :128 * (PT // 2)],
                                  mindT[:, :128 * (PT // 2)])
            if pt == 12:
                # first half of the logits accumulation (act_chain(0..7) done)
                psl = psum_l.tile([NB, C], F32, tag="psl",
                                  name=f"r{rep}psl")
                for lpt in range(PT // 2):
                    nc.tensor.matmul(
                        psl, act_bf[:, lpt], fcw_sb[:, lpt],
                        start=(lpt == 0), stop=False,
                    )
            if pt == PT - 2:
                act_chain(slice(PT // 2, PT - 1))

        # ---- tail: remaining transposes, act, logits ----
        for tpt in range(PT // 2, PT - 1):
            pst = psum_pool.tile([NB, 128], F32, tag="acc",
                                 name=f"r{rep}tp{tpt}")
            nc.tensor.transpose(pst, mind_sb[:, tpt], ident_sb[:])
            nc.scalar.copy(mindT[:, tpt * 128:(tpt + 1) * 128], pst)
        for lpt in range(PT // 2, PT - 1):
            nc.tensor.matmul(psl, act_bf[:, lpt], fcw_sb[:, lpt],
                             start=False, stop=False)
        pst = psum_pool.tile([NB, 128], F32, tag="acc", name=f"r{rep}tp_last")
        nc.tensor.transpose(pst, mind_sb[:, PT - 1], ident_sb[:])
        nc.scalar.copy(mindT[:, (PT - 1) * 128:PT * 128], pst)
        nc.sync.dma_start(mind[:, 128 * (PT // 2):],
                          mindT[:, 128 * (PT // 2):P])
        act_chain(slice(PT - 1, PT))
        nc.tensor.matmul(psl, act_bf[:, PT - 1], fcw_sb[:, PT - 1],
                         start=False, stop=True)
        logits_sb = misc.tile([NB, C], F32, tag="logits")
        nc.vector.tensor_copy(logits_sb[:], psl)
        nc.sync.dma_start(logits[:], logits_sb[:])


def build_program(reps=1):
    nc = bacc.Bacc("TRN2", target_bir_lowering=False, debug=False,
                   num_devices=NCORES)
    aps = {
        "patT": nc.dram_tensor("patT", [D, COLS], BF, kind="ExternalInput").ap(),
        "protT": nc.dram_tensor("protT", [D, PPAD], BF, kind="ExternalInput").ap(),
        "x2bc": nc.dram_tensor("x2bc", [128, COLS], F32, kind="ExternalInput").ap(),
        "p2col": nc.dram_tensor("p2col", [128, PT], F32, kind="ExternalInput").ap(),
        "fcwT": nc.dram_tensor("fcwT", [PPAD, C], BF, kind="ExternalInput").ap(),
        "ident": nc.dram_tensor("ident", [128, 128], F32, kind="ExternalInput").ap(),
        "attn": nc.dram_tensor("attn", [NB, P, N], F32, kind="ExternalOutput").ap(),
        "mind": nc.dram_tensor("mind", [NB, P], F32, kind="ExternalOutput").ap(),
        "logits": nc.dram_tensor("logits", [NB, C], F32, kind="ExternalOutput").ap(),
    }
    with tile.TileContext(nc) as tc:
        with ExitStack() as ctx:
            _emit(ctx, tc, aps, reps)
    nc.compile()
    return nc


def prep_inputs(patches, prototypes, fc_w):
    """Host-side shard + layout prep. Returns per-core input maps."""
    patches = np.asarray(patches, dtype=np.float32)
    prototypes = np.asarray(prototypes, dtype=np.float32)
    fc_w = np.asarray(fc_w, dtype=np.float32)

    # replicated tensors
    prot_pad = np.zeros((PPAD, D), dtype=np.float32)
    prot_pad[:P] = prototypes
    protT = np.ascontiguousarray(prot_pad.T).astype(BF16)          # [D, PPAD]
    p2 = np.sum(prototypes.astype(np.float64) ** 2, axis=1)
    p2_pad = np.zeros(PPAD)
    p2_pad[:P] = p2
    # p2col[pp, pt] = p2[pt*128 + pp], fp32 (used as ScalarE bias)
    p2col = np.ascontiguousarray(
        p2_pad.reshape(PT, 128).T).astype(np.float32)
    fcw_pad = np.zeros((PPAD, C), dtype=np.float32)
    fcw_pad[:P] = fc_w.T
    fcwT = fcw_pad.astype(BF16)
    ident = np.eye(128, dtype=np.float32)

    in_maps = []
    for c in range(NCORES):
        pat = patches[c * NB:(c + 1) * NB]                         # [8, 256, 1024]
        patT = np.ascontiguousarray(
            (-2.0 * pat).transpose(2, 0, 1).reshape(D, COLS)).astype(BF16)
        x2 = np.sum(pat.astype(np.float64) ** 2, axis=2).reshape(COLS)
        x2bc = np.ascontiguousarray(
            np.broadcast_to(x2.astype(np.float32), (128, COLS)))
        in_maps.append({
            "patT": patT, "protT": protT, "x2bc": x2bc, "p2col": p2col,
            "fcwT": fcwT, "ident": ident,
        })
    return in_maps


def assemble_outputs(results):
    logits = np.concatenate([r["logits"] for r in results], axis=0)
    attn = np.concatenate([r["attn"] for r in results], axis=0)
    mind = np.concatenate([r["mind"] for r in results], axis=0)
    return logits, attn, mind


_PROGRAM_CACHE = {}


def _get_program(reps=1):
    if reps not in _PROGRAM_CACHE:
        _PROGRAM_CACHE[reps] = build_program(reps)
    return _PROGRAM_CACHE[reps]


def kernel(patches, prototypes, fc_w):
    nc = _get_program(1)
    in_maps = prep_inputs(patches, prototypes, fc_w)
    res = run_bass_kernel_spmd(nc, in_maps, list(range(NCORES)))
    return assemble_outputs(res.results)
